# revision 7
# baseline (speedup 1.0000x reference)
import sys

sys.path.insert(0, "/opt/trn_rl_repo")
import numpy as np
from concourse import bass, bacc, mybir
import concourse.tile as tile
from concourse.bass_utils import run_bass_kernel_spmd

P = 128
D = 128
NC_N = 8
BLOCKS_PER_CORE = 98
NODES_PER_CORE = P * BLOCKS_PER_CORE      # 12544
NODES_PAD = NODES_PER_CORE * NC_N         # 100352
NUM_NODE = 100000
GCH = 32        # gather tiles per indirect DMA chunk
GBUFS = 6       # gather chunk buffers in flight

LAST_EXEC_NS = {}
LAST_NCS = {}


def _build_phase1():
    nc = bacc.Bacc()
    xT = nc.declare_dram_parameter("xT", [D, NODES_PER_CORE], mybir.dt.float32r, isOutput=False)
    wt = nc.declare_dram_parameter("wt", [D, D], mybir.dt.float32r, isOutput=False)
    ob = nc.declare_dram_parameter("ob", [2, D], mybir.dt.float32r, isOutput=False)
    y = nc.declare_dram_parameter("y", [NODES_PER_CORE, D], mybir.dt.float32r, isOutput=True)
    with tile.TileContext(nc) as tc:
        with tc.tile_pool(name="xt", bufs=1) as xp, \
             tc.tile_pool(name="w", bufs=1) as wp, \
             tc.tile_pool(name="yo", bufs=4) as yp, \
             tc.tile_pool(name="ps", bufs=4, space="PSUM") as pp:
            xt_t = xp.tile([D, NODES_PER_CORE], mybir.dt.float32r)
            NCH = 16
            cw = NODES_PER_CORE // NCH
            for c in range(NCH):
                nc.sync.dma_start(out=xt_t[:, c * cw:(c + 1) * cw],
                                  in_=xT[:, c * cw:(c + 1) * cw])
            wt_t = wp.tile([D, D], mybir.dt.float32r)
            nc.sync.dma_start(out=wt_t[:], in_=wt[:])
            ones_t = wp.tile([1, D], mybir.dt.float32r)
            nc.sync.dma_start(out=ones_t[:], in_=ob[0:1, :])
            b_t = wp.tile([1, D], mybir.dt.float32r)
            nc.sync.dma_start(out=b_t[:], in_=ob[1:2, :])
            for i in range(BLOCKS_PER_CORE):
                mm = pp.tile([P, D], mybir.dt.float32, space="PSUM")
                nc.tensor.matmul(out=mm[:], lhsT=xt_t[:, i * P:(i + 1) * P],
                                 rhs=wt_t[:], start=True, stop=False)
                nc.tensor.matmul(out=mm[:], lhsT=ones_t[:], rhs=b_t[:],
                                 start=False, stop=True)
                y_sb = yp.tile([P, D], mybir.dt.float32r)
                nc.scalar.activation(out=y_sb[:], in_=mm[:],
                                     func=mybir.ActivationFunctionType.Relu)
                nc.sync.dma_start(out=y[i * P:(i + 1) * P, :], in_=y_sb[:])
    nc.finalize()
    return nc


def _build_phase2(T, block_of_tile, tile_start):
    nc = bacc.Bacc()
    g = nc.declare_dram_parameter("g", [P, T * D], mybir.dt.float32r, isOutput=False)
    slf = nc.declare_dram_parameter("slf", [P, T], mybir.dt.float32, isOutput=False)
    rdeg = nc.declare_dram_parameter("rdeg", [P, BLOCKS_PER_CORE], mybir.dt.float32, isOutput=False)
    out = nc.declare_dram_parameter("out", [NODES_PER_CORE, D], mybir.dt.float32, isOutput=True)
    with tile.TileContext(nc) as tc:
        with tc.tile_pool(name="mt", bufs=1) as mt, \
             tc.tile_pool(name="gb", bufs=GBUFS) as gbp, \
             tc.tile_pool(name="sp", bufs=6) as spool, \
             tc.tile_pool(name="op", bufs=4) as opool, \
             tc.tile_pool(name="ps", bufs=4, space="PSUM") as pp:
            slf_t = mt.tile([P, T], mybir.dt.float32)
            nc.sync.dma_start(out=slf_t[:], in_=slf[:])
            rd_t = mt.tile([P, BLOCKS_PER_CORE], mybir.dt.float32)
            nc.sync.dma_start(out=rd_t[:], in_=rdeg[:])
            iota_i = mt.tile([P, P], mybir.dt.int32)
            nc.gpsimd.iota(iota_i[:], pattern=[[1, P]], base=0, channel_multiplier=0)
            iota_f = mt.tile([P, P], mybir.dt.float32)
            nc.vector.tensor_copy(iota_f[:], iota_i[:])

            gb = None
            mm = None
            for t in range(T):
                if t % GCH == 0:
                    n = min(GCH, T - t)
                    gb = gbp.tile([P, n * D], mybir.dt.float32r)
                    nc.sync.dma_start(out=gb[:], in_=g[:, t * D:(t + n) * D])
                j = int(block_of_tile[t])
                first = (t == tile_start[j])
                last = (t == tile_start[j + 1] - 1)
                s_f = spool.tile([P, P], mybir.dt.float32)
                nc.vector.tensor_tensor(
                    out=s_f[:],
                    in0=slf_t[:, t:t + 1].to_broadcast([P, P])[:],
                    in1=iota_f[:],
                    op=mybir.AluOpType.is_equal,
                )
                s_t = spool.tile([P, P], mybir.dt.float32r)
                nc.scalar.activation(out=s_t[:], in_=s_f[:],
                                     func=mybir.ActivationFunctionType.Copy,
                                     bias=0.0, scale=1.0)
                if first:
                    mm = pp.tile([P, D], mybir.dt.float32, space="PSUM")
                c = t % GCH
                nc.tensor.matmul(out=mm[:], lhsT=s_t[:], rhs=gb[:, c * D:(c + 1) * D],
                                 start=first, stop=last)
                if last:
                    o_t = opool.tile([P, D], mybir.dt.float32)
                    nc.scalar.activation(out=o_t[:], in_=mm[:],
                                         func=mybir.ActivationFunctionType.Copy,
                                         bias=0.0, scale=rd_t[:, j:j + 1])
                    nc.sync.dma_start(out=out[j * P:(j + 1) * P, :], in_=o_t[:])
    nc.finalize()
    return nc


def kernel(**inputs):
    x = np.asarray(inputs["x"], np.float32)
    ei = np.asarray(inputs["edge_index"])
    W = np.asarray(inputs["W"], np.float32)
    b = np.asarray(inputs["b"], np.float32)
    src = ei[0].astype(np.int64)
    tgt = ei[1].astype(np.int64)
    E = src.shape[0]

    order = np.argsort(src, kind="stable")
    ss = src[order]
    ts = tgt[order].astype(np.int32)
    bounds = np.searchsorted(ss, np.arange(0, NODES_PAD + 1, P)).astype(np.int64)
    cnt = (bounds[1:] - bounds[:-1]).reshape(NC_N, BLOCKS_PER_CORE)
    Kb = np.maximum(1, (cnt.max(axis=0) + P - 1) // P)
    tile_start = np.concatenate([[0], np.cumsum(Kb)]).astype(np.int64)
    T = int(tile_start[-1])
    block_of_tile = np.repeat(np.arange(BLOCKS_PER_CORE), Kb)

    jj = ss // P
    m = np.arange(E, dtype=np.int64) - bounds[jj]
    p = m % P
    trel = m // P
    k = jj // BLOCKS_PER_CORE
    jloc = jj % BLOCKS_PER_CORE
    tglob = tile_start[jloc] + trel
    tg_arr = np.zeros((NC_N, P, T), np.int32)
    sl_arr = np.full((NC_N, P, T), 255.0, np.float32)
    tg_arr[k, p, tglob] = ts
    sl_arr[k, p, tglob] = (ss % P).astype(np.float32)

    deg = np.bincount(src, minlength=NODES_PAD).astype(np.float32)
    rdeg = 1.0 / np.maximum(deg, 1.0)
    rdeg_arr = np.ascontiguousarray(
        rdeg.reshape(NC_N, BLOCKS_PER_CORE, P).transpose(0, 2, 1))

    x_pad = np.zeros((NODES_PAD, D), np.float32)
    x_pad[:NUM_NODE] = x
    wt = np.ascontiguousarray(W.T)
    ob = np.stack([np.ones(D, np.float32), b])

    nc1 = _build_phase1()
    in1 = []
    for kk in range(NC_N):
        xs = np.ascontiguousarray(
            x_pad[kk * NODES_PER_CORE:(kk + 1) * NODES_PER_CORE].T)
        in1.append({"xT": xs, "wt": wt, "ob": ob})
    res1 = run_bass_kernel_spmd(nc1, in1, list(range(NC_N)))
    LAST_EXEC_NS["phase1"] = res1.exec_time_ns
    LAST_NCS["phase1"] = nc1
    y_full = np.ascontiguousarray(
        np.concatenate([res1.results[kk]["y"] for kk in range(NC_N)], axis=0))

    nc2 = _build_phase2(T, block_of_tile, tile_start)
    in2 = []
    for kk in range(NC_N):
        g_arr = np.ascontiguousarray(y_full[tg_arr[kk]].reshape(P, T * D))
        in2.append({"g": g_arr, "slf": sl_arr[kk], "rdeg": rdeg_arr[kk]})
    res2 = run_bass_kernel_spmd(nc2, in2, list(range(NC_N)))
    LAST_EXEC_NS["phase2"] = res2.exec_time_ns
    LAST_NCS["phase2"] = nc2
    out = np.concatenate([res2.results[kk]["out"] for kk in range(NC_N)], axis=0)
    return np.ascontiguousarray(out[:NUM_NODE]).astype(np.float32)


# revision 8
# speedup vs baseline: 1.4247x; 1.4247x over previous
import sys

sys.path.insert(0, "/opt/trn_rl_repo")
import numpy as np
import ml_dtypes
from concourse import bass, bacc, mybir
import concourse.tile as tile
from concourse.bass_utils import run_bass_kernel_spmd

BF = ml_dtypes.bfloat16
P = 128
D = 128
NC_N = 8
BLOCKS_PER_CORE = 98
NODES_PER_CORE = P * BLOCKS_PER_CORE      # 12544
NODES_PAD = NODES_PER_CORE * NC_N         # 100352
NUM_NODE = 100000
GCH = 16        # gather tiles per chunk (DMA + one DVE S-build op)
GBUFS = 5

LAST_EXEC_NS = {}
LAST_NCS = {}


def _build_phase1():
    nc = bacc.Bacc()
    xT = nc.declare_dram_parameter("xT", [D, NODES_PER_CORE], mybir.dt.float32r, isOutput=False)
    wt = nc.declare_dram_parameter("wt", [D, D], mybir.dt.float32r, isOutput=False)
    ob = nc.declare_dram_parameter("ob", [2, D], mybir.dt.float32r, isOutput=False)
    y = nc.declare_dram_parameter("y", [NODES_PER_CORE, D], mybir.dt.float32, isOutput=True)
    with tile.TileContext(nc) as tc:
        with tc.tile_pool(name="xt", bufs=1) as xp, \
             tc.tile_pool(name="w", bufs=1) as wp, \
             tc.tile_pool(name="yo", bufs=3) as yp, \
             tc.tile_pool(name="ps", bufs=4, space="PSUM") as pp:
            xt_t = xp.tile([D, NODES_PER_CORE], mybir.dt.float32r)
            NCH = 16
            cw = NODES_PER_CORE // NCH
            for c in range(NCH):
                nc.sync.dma_start(out=xt_t[:, c * cw:(c + 1) * cw],
                                  in_=xT[:, c * cw:(c + 1) * cw])
            wt_t = wp.tile([D, D], mybir.dt.float32r)
            nc.sync.dma_start(out=wt_t[:], in_=wt[:])
            ones_t = wp.tile([1, D], mybir.dt.float32r)
            nc.sync.dma_start(out=ones_t[:], in_=ob[0:1, :])
            b_t = wp.tile([1, D], mybir.dt.float32r)
            nc.sync.dma_start(out=b_t[:], in_=ob[1:2, :])
            o_all = None
            for i in range(BLOCKS_PER_CORE):
                mm = pp.tile([P, D], mybir.dt.float32, space="PSUM")
                nc.tensor.matmul(out=mm[:], lhsT=xt_t[:, i * P:(i + 1) * P],
                                 rhs=wt_t[:], start=True, stop=False)
                nc.tensor.matmul(out=mm[:], lhsT=ones_t[:], rhs=b_t[:],
                                 start=False, stop=True)
                bi = i % 8
                if bi == 0:
                    o_all = yp.tile([P, 8, D], mybir.dt.float32)
                nc.scalar.activation(out=o_all[:, bi, :], in_=mm[:],
                                     func=mybir.ActivationFunctionType.Relu)
                if bi == 7 or i == BLOCKS_PER_CORE - 1:
                    i0 = i - bi
                    nc.sync.dma_start(
                        out=y[i0 * P:(i + 1) * P, :].rearrange("(c p) d -> p c d", p=P),
                        in_=o_all[:, 0:bi + 1, :])
    nc.finalize()
    return nc


def _build_phase2(T, block_of_tile, tile_start):
    nc = bacc.Bacc()
    g = nc.declare_dram_parameter("g", [P, T * 2 * D], mybir.dt.bfloat16, isOutput=False)
    slf = nc.declare_dram_parameter("slf", [P, T], mybir.dt.bfloat16, isOutput=False)
    rdeg = nc.declare_dram_parameter("rdeg", [P, BLOCKS_PER_CORE], mybir.dt.float32, isOutput=False)
    out = nc.declare_dram_parameter("out", [NODES_PER_CORE, D], mybir.dt.float32, isOutput=True)
    with tile.TileContext(nc) as tc:
        with tc.tile_pool(name="mt", bufs=1) as mt, \
             tc.tile_pool(name="gb", bufs=GBUFS) as gbp, \
             tc.tile_pool(name="sp", bufs=GBUFS) as spool, \
             tc.tile_pool(name="op", bufs=3) as opool, \
             tc.tile_pool(name="ps", bufs=4, space="PSUM") as pp:
            slf_t = mt.tile([P, T], mybir.dt.bfloat16)
            nc.sync.dma_start(out=slf_t[:], in_=slf[:])
            rd_t = mt.tile([P, BLOCKS_PER_CORE], mybir.dt.float32)
            nc.sync.dma_start(out=rd_t[:], in_=rdeg[:])
            iota_i = mt.tile([P, P], mybir.dt.int32)
            nc.gpsimd.iota(iota_i[:], pattern=[[1, P]], base=0, channel_multiplier=0)
            iota_b = mt.tile([P, 1, P], mybir.dt.bfloat16)
            nc.vector.tensor_copy(iota_b[:, 0, :], iota_i[:])

            gb = None
            s_all = None
            mm = None
            o_all = None
            for t in range(T):
                c = t % GCH
                if c == 0:
                    n = min(GCH, T - t)
                    gb = gbp.tile([P, n * 2 * D], mybir.dt.bfloat16)
                    nc.sync.dma_start(out=gb[:], in_=g[:, t * 2 * D:(t + n) * 2 * D])
                    s_all = spool.tile([P, n, P], mybir.dt.bfloat16)
                    nc.vector.tensor_tensor(
                        out=s_all[:],
                        in0=slf_t[:, t:t + n].to_broadcast([P, n, P])[:],
                        in1=iota_b[:].to_broadcast([P, n, P])[:],
                        op=mybir.AluOpType.is_equal,
                    )
                j = int(block_of_tile[t])
                first = (t == tile_start[j])
                last = (t == tile_start[j + 1] - 1)
                if first:
                    mm = pp.tile([P, D], mybir.dt.float32, space="PSUM")
                nc.tensor.matmul(out=mm[:], lhsT=s_all[:, c, :],
                                 rhs=gb[:, (2 * c) * D:(2 * c + 1) * D],
                                 start=first, stop=False)
                nc.tensor.matmul(out=mm[:], lhsT=s_all[:, c, :],
                                 rhs=gb[:, (2 * c + 1) * D:(2 * c + 2) * D],
                                 start=False, stop=last)
                if last:
                    bi = j % 8
                    if bi == 0:
                        o_all = opool.tile([P, 8, D], mybir.dt.float32)
                    nc.scalar.activation(out=o_all[:, bi, :], in_=mm[:],
                                         func=mybir.ActivationFunctionType.Copy,
                                         bias=0.0, scale=rd_t[:, j:j + 1])
                    if bi == 7 or j == BLOCKS_PER_CORE - 1:
                        j0 = j - bi
                        nc.sync.dma_start(
                            out=out[j0 * P:(j + 1) * P, :].rearrange("(c p) d -> p c d", p=P),
                            in_=o_all[:, 0:bi + 1, :])
    nc.finalize()
    return nc


def kernel(**inputs):
    x = np.asarray(inputs["x"], np.float32)
    ei = np.asarray(inputs["edge_index"])
    W = np.asarray(inputs["W"], np.float32)
    b = np.asarray(inputs["b"], np.float32)
    src = ei[0].astype(np.int64)
    tgt = ei[1].astype(np.int64)
    E = src.shape[0]

    order = np.argsort(src, kind="stable")
    ss = src[order]
    ts = tgt[order].astype(np.int32)
    bounds = np.searchsorted(ss, np.arange(0, NODES_PAD + 1, P)).astype(np.int64)
    cnt = (bounds[1:] - bounds[:-1]).reshape(NC_N, BLOCKS_PER_CORE)
    Kb = np.maximum(1, (cnt.max(axis=0) + P - 1) // P)
    tile_start = np.concatenate([[0], np.cumsum(Kb)]).astype(np.int64)
    T = int(tile_start[-1])
    block_of_tile = np.repeat(np.arange(BLOCKS_PER_CORE), Kb)

    jj = ss // P
    m = np.arange(E, dtype=np.int64) - bounds[jj]
    p = m % P
    trel = m // P
    k = jj // BLOCKS_PER_CORE
    jloc = jj % BLOCKS_PER_CORE
    tglob = tile_start[jloc] + trel
    tg_arr = np.zeros((NC_N, P, T), np.int32)
    sl_arr = np.full((NC_N, P, T), 255.0, np.float32)
    tg_arr[k, p, tglob] = ts
    sl_arr[k, p, tglob] = (ss % P).astype(np.float32)
    sl_arr = sl_arr.astype(BF)

    deg = np.bincount(src, minlength=NODES_PAD).astype(np.float32)
    rdeg = 1.0 / np.maximum(deg, 1.0)
    rdeg_arr = np.ascontiguousarray(
        rdeg.reshape(NC_N, BLOCKS_PER_CORE, P).transpose(0, 2, 1))

    x_pad = np.zeros((NODES_PAD, D), np.float32)
    x_pad[:NUM_NODE] = x
    wt = np.ascontiguousarray(W.T)
    ob = np.stack([np.ones(D, np.float32), b])

    nc1 = _build_phase1()
    in1 = []
    for kk in range(NC_N):
        xs = np.ascontiguousarray(
            x_pad[kk * NODES_PER_CORE:(kk + 1) * NODES_PER_CORE].T)
        in1.append({"xT": xs, "wt": wt, "ob": ob})
    res1 = run_bass_kernel_spmd(nc1, in1, list(range(NC_N)))
    LAST_EXEC_NS["phase1"] = res1.exec_time_ns
    LAST_NCS["phase1"] = nc1
    y_full = np.ascontiguousarray(
        np.concatenate([res1.results[kk]["y"] for kk in range(NC_N)], axis=0))

    y_hi = y_full.astype(BF)
    y_lo = (y_full - y_hi.astype(np.float32)).astype(BF)

    nc2 = _build_phase2(T, block_of_tile, tile_start)
    in2 = []
    for kk in range(NC_N):
        tg = tg_arr[kk]
        g_arr = np.empty((P, T, 2, D), BF)
        g_arr[:, :, 0, :] = y_hi[tg]
        g_arr[:, :, 1, :] = y_lo[tg]
        in2.append({"g": g_arr.reshape(P, T * 2 * D),
                    "slf": sl_arr[kk], "rdeg": rdeg_arr[kk]})
    res2 = run_bass_kernel_spmd(nc2, in2, list(range(NC_N)))
    LAST_EXEC_NS["phase2"] = res2.exec_time_ns
    LAST_NCS["phase2"] = nc2
    out = np.concatenate([res2.results[kk]["out"] for kk in range(NC_N)], axis=0)
    return np.ascontiguousarray(out[:NUM_NODE]).astype(np.float32)


# revision 17
# speedup vs baseline: 1.8606x; 1.3059x over previous
import sys

sys.path.insert(0, "/opt/trn_rl_repo")
import numpy as np
import ml_dtypes
from concourse import bass, bacc, mybir
import concourse.tile as tile
from concourse.bass_utils import run_bass_kernel_spmd

BF = ml_dtypes.bfloat16
P = 128
D = 128
NC_N = 8
BLOCKS_PER_CORE = 98
NODES_PER_CORE = P * BLOCKS_PER_CORE      # 12544
NODES_PAD = NODES_PER_CORE * NC_N         # 100352
NUM_NODE = 100000
GCH = 22        # gather tiles per chunk (DMA + one DVE S-build op)
GBUFS = 8

LAST_EXEC_NS = {}
LAST_NCS = {}


GW = 512          # matmul group width (nodes per wide matmul, 1 PSUM bank)
OGRP = 4          # groups per output buffer / DMA


def _build_phase1():
    nc = bacc.Bacc()
    xT = nc.declare_dram_parameter("xT", [D, NODES_PER_CORE], mybir.dt.float32r, isOutput=False)
    wt = nc.declare_dram_parameter("wt", [D, D], mybir.dt.float32r, isOutput=False)
    bc = nc.declare_dram_parameter("bc", [D, 1], mybir.dt.float32, isOutput=False)
    yT = nc.declare_dram_parameter("yT", [D, NODES_PER_CORE], mybir.dt.float32, isOutput=True)
    with tile.TileContext(nc) as tc:
        with tc.tile_pool(name="xt", bufs=1) as xp, \
             tc.tile_pool(name="w", bufs=1) as wp, \
             tc.tile_pool(name="yo", bufs=3) as yp, \
             tc.tile_pool(name="ps", bufs=4, space="PSUM") as pp:
            wt_t = wp.tile([D, D], mybir.dt.float32r)
            nc.sync.dma_start(out=wt_t[:], in_=wt[:])
            bc_t = wp.tile([D, 1], mybir.dt.float32)
            nc.sync.dma_start(out=bc_t[:], in_=bc[:])
            xt_t = xp.tile([D, NODES_PER_CORE], mybir.dt.float32r)
            NCH = 16
            cw = NODES_PER_CORE // NCH
            for c in range(NCH):
                nc.sync.dma_start(out=xt_t[:, c * cw:(c + 1) * cw],
                                  in_=xT[:, c * cw:(c + 1) * cw])
            ngrp = (NODES_PER_CORE + GW - 1) // GW
            o_all = None
            o0 = 0
            for gi in range(ngrp):
                c0 = gi * GW
                w_ = min(GW, NODES_PER_CORE - c0)
                mm = pp.tile([D, w_], mybir.dt.float32, space="PSUM")
                nc.tensor.matmul(out=mm[:], lhsT=wt_t[:],
                                 rhs=xt_t[:, c0:c0 + w_], start=True, stop=True)
                if gi % OGRP == 0:
                    o_all = yp.tile([D, OGRP * GW], mybir.dt.float32)
                    o0 = c0
                nc.scalar.activation(out=o_all[:, c0 - o0:c0 - o0 + w_], in_=mm[:],
                                     func=mybir.ActivationFunctionType.Relu,
                                     bias=bc_t[:, 0:1], scale=1.0)
                if gi % OGRP == OGRP - 1 or gi == ngrp - 1:
                    nc.sync.dma_start(out=yT[:, o0:c0 + w_],
                                      in_=o_all[:, 0:c0 + w_ - o0])
    nc.finalize()
    return nc


def _build_phase2(T, block_of_tile, tile_start, gch=GCH, gbufs=GBUFS,
                  sbufs=None, obufs=4, pbufs=6):
    if sbufs is None:
        sbufs = gbufs
    nc = bacc.Bacc()
    g = nc.declare_dram_parameter("g", [P, T * 2 * D], mybir.dt.bfloat16, isOutput=False)
    slf = nc.declare_dram_parameter("slf", [P, T], mybir.dt.bfloat16, isOutput=False)
    rdeg = nc.declare_dram_parameter("rdeg", [P, BLOCKS_PER_CORE], mybir.dt.float32, isOutput=False)
    out = nc.declare_dram_parameter("out", [NODES_PER_CORE, D], mybir.dt.float32, isOutput=True)
    with tile.TileContext(nc) as tc:
        with tc.tile_pool(name="mt", bufs=1) as mt, \
             tc.tile_pool(name="gb", bufs=gbufs) as gbp, \
             tc.tile_pool(name="sp", bufs=sbufs) as spool, \
             tc.tile_pool(name="op", bufs=obufs) as opool, \
             tc.tile_pool(name="ps", bufs=pbufs, space="PSUM") as pp:
            slf_t = mt.tile([P, T], mybir.dt.bfloat16)
            nc.sync.dma_start(out=slf_t[:], in_=slf[:])
            rd_t = mt.tile([P, BLOCKS_PER_CORE], mybir.dt.float32)
            nc.sync.dma_start(out=rd_t[:], in_=rdeg[:])
            iota_i = mt.tile([P, P], mybir.dt.int32)
            nc.gpsimd.iota(iota_i[:], pattern=[[1, P]], base=0, channel_multiplier=0)
            iota_b = mt.tile([P, 1, P], mybir.dt.bfloat16)
            nc.vector.tensor_copy(iota_b[:, 0, :], iota_i[:])

            gb = None
            s_all = None
            mm = None
            o_all = None
            for t in range(T):
                c = t % gch
                if c == 0:
                    n = min(gch, T - t)
                    gb = gbp.tile([P, n * 2 * D], mybir.dt.bfloat16)
                    nc.sync.dma_start(out=gb[:], in_=g[:, t * 2 * D:(t + n) * 2 * D])
                    s_all = spool.tile([P, n, P], mybir.dt.bfloat16)
                    nc.vector.tensor_tensor(
                        out=s_all[:],
                        in0=slf_t[:, t:t + n].to_broadcast([P, n, P])[:],
                        in1=iota_b[:].to_broadcast([P, n, P])[:],
                        op=mybir.AluOpType.is_equal,
                    )
                j = int(block_of_tile[t])
                first = (t == tile_start[j])
                last = (t == tile_start[j + 1] - 1)
                if first:
                    mm = pp.tile([P, D], mybir.dt.float32, space="PSUM")
                nc.tensor.matmul(out=mm[:], lhsT=s_all[:, c, :],
                                 rhs=gb[:, (2 * c) * D:(2 * c + 1) * D],
                                 start=first, stop=False)
                nc.tensor.matmul(out=mm[:], lhsT=s_all[:, c, :],
                                 rhs=gb[:, (2 * c + 1) * D:(2 * c + 2) * D],
                                 start=False, stop=last)
                if last:
                    bi = j % 8
                    if bi == 0:
                        o_all = opool.tile([P, 8, D], mybir.dt.float32)
                    nc.scalar.activation(out=o_all[:, bi, :], in_=mm[:],
                                         func=mybir.ActivationFunctionType.Copy,
                                         bias=0.0, scale=rd_t[:, j:j + 1])
                    if bi == 7 or j == BLOCKS_PER_CORE - 1:
                        j0 = j - bi
                        nc.sync.dma_start(
                            out=out[j0 * P:(j + 1) * P, :].rearrange("(c p) d -> p c d", p=P),
                            in_=o_all[:, 0:bi + 1, :])
    nc.finalize()
    return nc


def kernel(**inputs):
    x = np.asarray(inputs["x"], np.float32)
    ei = np.asarray(inputs["edge_index"])
    W = np.asarray(inputs["W"], np.float32)
    b = np.asarray(inputs["b"], np.float32)
    src = ei[0].astype(np.int64)
    tgt = ei[1].astype(np.int64)
    E = src.shape[0]

    order = np.argsort(src, kind="stable")
    ss = src[order]
    ts = tgt[order].astype(np.int32)
    bounds = np.searchsorted(ss, np.arange(0, NODES_PAD + 1, P)).astype(np.int64)
    cnt = (bounds[1:] - bounds[:-1]).reshape(NC_N, BLOCKS_PER_CORE)
    # per-core block permutation: sort blocks by edge count so per-slot
    # maxima across cores align, minimizing padded tile count T
    pi = np.argsort(-cnt, axis=1, kind="stable")
    inv = np.argsort(pi, axis=1)
    cnt_sorted = np.take_along_axis(cnt, pi, axis=1)
    Kb = np.maximum(1, (cnt_sorted.max(axis=0) + P - 1) // P)
    tile_start = np.concatenate([[0], np.cumsum(Kb)]).astype(np.int64)
    T = int(tile_start[-1])
    block_of_tile = np.repeat(np.arange(BLOCKS_PER_CORE), Kb)

    jj = ss // P
    m = np.arange(E, dtype=np.int64) - bounds[jj]
    p = m % P
    trel = m // P
    k = jj // BLOCKS_PER_CORE
    jloc = jj % BLOCKS_PER_CORE
    r = inv[k, jloc]
    tglob = tile_start[r] + trel
    tg_arr = np.zeros((NC_N, P, T), np.int32)
    sl_arr = np.full((NC_N, P, T), 255.0, np.float32)
    tg_arr[k, p, tglob] = ts
    sl_arr[k, p, tglob] = (ss % P).astype(np.float32)
    sl_arr = sl_arr.astype(BF)

    deg = np.bincount(src, minlength=NODES_PAD).astype(np.float32)
    rdeg = 1.0 / np.maximum(deg, 1.0)
    rdeg_resh = rdeg.reshape(NC_N, BLOCKS_PER_CORE, P)
    rdeg_slot = np.take_along_axis(rdeg_resh, pi[:, :, None], axis=1)
    rdeg_arr = np.ascontiguousarray(rdeg_slot.transpose(0, 2, 1))

    x_pad = np.zeros((NODES_PAD, D), np.float32)
    x_pad[:NUM_NODE] = x
    wt = np.ascontiguousarray(W.T)

    nc1 = _build_phase1()
    in1 = []
    for kk in range(NC_N):
        xs = np.ascontiguousarray(
            x_pad[kk * NODES_PER_CORE:(kk + 1) * NODES_PER_CORE].T)
        in1.append({"xT": xs, "wt": wt, "bc": b.reshape(D, 1)})
    res1 = run_bass_kernel_spmd(nc1, in1, list(range(NC_N)))
    LAST_EXEC_NS["phase1"] = res1.exec_time_ns
    LAST_NCS["phase1"] = nc1
    y_full = np.ascontiguousarray(
        np.concatenate([res1.results[kk]["yT"].T for kk in range(NC_N)], axis=0))

    y_hi = y_full.astype(BF)
    y_lo = (y_full - y_hi.astype(np.float32)).astype(BF)

    nc2 = _build_phase2(T, block_of_tile, tile_start)
    in2 = []
    for kk in range(NC_N):
        tg = tg_arr[kk]
        g_arr = np.empty((P, T, 2, D), BF)
        g_arr[:, :, 0, :] = y_hi[tg]
        g_arr[:, :, 1, :] = y_lo[tg]
        in2.append({"g": g_arr.reshape(P, T * 2 * D),
                    "slf": sl_arr[kk], "rdeg": rdeg_arr[kk]})
    res2 = run_bass_kernel_spmd(nc2, in2, list(range(NC_N)))
    LAST_EXEC_NS["phase2"] = res2.exec_time_ns
    LAST_NCS["phase2"] = nc2
    parts = []
    for kk in range(NC_N):
        outk = res2.results[kk]["out"].reshape(BLOCKS_PER_CORE, P, D)
        unperm = np.empty_like(outk)
        unperm[pi[kk]] = outk
        parts.append(unperm.reshape(NODES_PER_CORE, D))
    out = np.concatenate(parts, axis=0)
    return np.ascontiguousarray(out[:NUM_NODE]).astype(np.float32)
